# revision 16
# baseline (speedup 1.0000x reference)
"""MQA kernel for Trainium2 (8 NeuronCores, SPMD via bass/Tile).

Problem: nn_MultiQueryAttention (B=2, T=2048, HID=2048, H=16, D=128).

Key algebraic simplification: the reference's apply_rope treats q's layout
as (B,T,H,D) while q is actually (B,H,T,D), so the "position" axis is the
head index -> per-head rotation R_h acting on the D axis only, independent
of sequence position. R_h is folded into Wq on the host. k's rope at pos=0
is a pure channel permutation, folded into Wk. The score scale 1/sqrt(D)
is folded into Wq as well. What remains on-device is a plain causal MQA.

Sharding (uniform SPMD program, per-core data differs):
  core c -> batch c//4, heads (c%4)*4..(c%4)*4+3, full T.
  Each core: Q^T/K^T/V projections, causal softmax attention for its 4
  heads, and a partial out-projection (its heads' rows of Wo^T). The 4
  partials per batch are summed on the host.

All matmuls run as float32r (full fp32 data, fast PE mode), fp32 PSUM
accumulation. bf16 was tried and is a net loss on this part: the
compiler-automatic FWL weight loads contend with the streaming operand
(matmul spacing 259ns vs 231ns) and ACT/DVE slow down ~20%.

Structure (vs the 348us baseline; measured 284.6us):
  - Softmax denominator matmul uses a [128,128] all-ones lhsT so the
    denominator lands replicated across partitions: the rank-1 broadcast
    matmul and the 1-partition DVE chain disappear.
  - Attention for query tile tb emits scores in order [0, 1, diag, 2,
    ...], den/ot matmuls two behind, diag den/ot last: the diagonal
    exp+mask chain and the ACT latency hide under the score stream.
  - Out-projection for tile tb is emitted after tile tb+1's attention
    loop, so the recip/normalize DVE chain hides under out-proj matmuls.
  - DMA: hst arrives in batched chunks on the sync HWDGE ring while
    weights stream just-in-time on the ACT HWDGE ring (per-DMA fixed
    costs serialize within a ring, capping many-small-DMA streams at
    ~200GB/s); K/V projection matmuls lag the Q matmuls by 3 k-steps so
    the wk/wv arrival is off the critical path; per-block V-transposes
    are deferred into the next block's k-loop; Wo loads spread across
    t-blocks 1-2; exp activation table preloaded during phase 1.
"""

import numpy as np
from contextlib import ExitStack

import concourse.bass as bass
import concourse.tile as tile
from concourse import bacc, mybir
from concourse.bass_utils import run_bass_kernel_spmd
from concourse.masks import make_identity

F32 = mybir.dt.float32
F32R = mybir.dt.float32r
EXP = mybir.ActivationFunctionType.Exp

B, T, HID, H, D = 2, 2048, 2048, 16, 128
NCORES = 8
CPB = 4              # cores per batch
HPC = H // CPB       # 4 heads per core
HD_PC = HPC * D      # 512 output dims per core
P = 128
KT = T // P          # 16 key tiles
NK = HID // P        # 16 contraction tiles for projections


def _rope_fold():
    """Per-head rotation matrices R_h (128x128) from the reference's quirky rope."""
    half = D // 2
    theta = 1.0 / (10000.0 ** (np.arange(0, half, 2, dtype=np.float64) / half))
    mats = []
    for h in range(H):
        R = np.zeros((D, D), dtype=np.float64)
        c = np.cos(h * theta)
        s = np.sin(h * theta)
        for j in range(32):
            R[j, 2 * j] = c[j]
            R[j, 2 * j + 1] = -s[j]
            R[32 + j, 2 * j] = s[j]
            R[32 + j, 2 * j + 1] = c[j]
            R[64 + j, 64 + 2 * j] = c[j]
            R[64 + j, 64 + 2 * j + 1] = -s[j]
            R[96 + j, 64 + 2 * j] = s[j]
            R[96 + j, 64 + 2 * j + 1] = c[j]
        mats.append(R)
    return mats


def _build_program():
    nc = bacc.Bacc("TRN2", target_bir_lowering=False, debug=False,
                   enable_asserts=False, num_devices=NCORES)

    hsT = nc.dram_tensor("hsT", [HID, T], F32R, kind="ExternalInput").ap()
    wqT = nc.dram_tensor("wqT", [HID, HD_PC], F32R, kind="ExternalInput").ap()
    wkT = nc.dram_tensor("wkT", [HID, D], F32R, kind="ExternalInput").ap()
    wvT = nc.dram_tensor("wvT", [HID, D], F32R, kind="ExternalInput").ap()
    woT = nc.dram_tensor("woT", [HD_PC, HID], F32R, kind="ExternalInput").ap()
    dmd = nc.dram_tensor("dmask", [P, P], F32R, kind="ExternalInput").ap()
    onesd = nc.dram_tensor("onesd", [P, P], F32R, kind="ExternalInput").ap()
    out = nc.dram_tensor("out", [T, HID], F32, kind="ExternalOutput").ap()

    hsT_r = hsT.rearrange("(ko p) t -> p ko t", p=P)        # [128,16,2048]
    wqT_r = wqT.rearrange("(ko p) m -> p ko m", p=P)        # [128,16,512]
    wkT_r = wkT.rearrange("(ko p) d -> p ko d", p=P)        # [128,16,128]
    wvT_r = wvT.rearrange("(ko p) d -> p ko d", p=P)
    woT_r = woT.rearrange("(h p) n -> p h n", p=P)          # [128,4,2048]
    out_r = out.rearrange("(tt p) n -> tt p n", p=P)        # [16,128,2048]

    def mm(ps, lhsT, rhs, start, stop):
        nc.tensor.matmul(ps, lhsT=lhsT, rhs=rhs, start=start, stop=stop)

    with tile.TileContext(nc) as tc, ExitStack() as ctx:
        singles = ctx.enter_context(tc.tile_pool(name="singles", bufs=1))
        hpool = ctx.enter_context(tc.tile_pool(name="hst", bufs=3))
        epool = ctx.enter_context(tc.tile_pool(name="etile", bufs=6))
        spool = ctx.enter_context(tc.tile_pool(name="small", bufs=2))
        apool = ctx.enter_context(tc.tile_pool(name="att", bufs=3))
        opool = ctx.enter_context(tc.tile_pool(name="outt", bufs=3))

        ident = singles.tile([P, P], F32)
        make_identity(nc, ident)
        dmask = singles.tile([P, P], F32R)
        ones = singles.tile([P, P], F32R)
        # preload the exp activation table while phase 1 streams
        warm = spool.tile([1, 1], F32, tag="warm")
        nc.scalar.activation(warm[:], ident[:1, :1], EXP)

        # weight residents; per-k slices are DMA'd inside the first
        # phase-1 block so the first matmuls start after ~1us; the
        # out-projection weights load during blocks 1-2.
        wq_sb = singles.tile([P, NK, HD_PC], F32R)
        wk_sb = singles.tile([P, NK, D], F32R)
        wv_sb = singles.tile([P, NK, D], F32R)
        wo_sb = singles.tile([P, HPC, HID], F32R)

        # resident activations
        qt_sb = singles.tile([P, HPC, T], F32R)      # Q^T per head [d, t]
        kt_sb = singles.tile([P, T], F32R)           # K^T [d, s]
        v_sb = singles.tile([P, KT, D], F32R)        # V natural [s-tile, d]

        # ---------------- Phase 1: Q/K/V projections ----------------
        # K/V matmuls run 3 k-steps behind the Q matmuls so the initial
        # wk/wv weight DMAs are off the critical path; each block's
        # V-transposes are deferred into the next block's k-loop.
        KVLAG = 3

        def transpose_v(tb4, vt):
            for si in range(4):
                pt = ps1t.tile([P, P], F32, tag="tps")
                nc.tensor.transpose(pt[:], vt[:, si * P:(si + 1) * P], ident[:])
                nc.vector.tensor_copy(v_sb[:, tb4 * 4 + si, :], pt[:])

        with tc.tile_pool(name="ps1", bufs=1, space="PSUM") as ps1, \
             tc.tile_pool(name="ps1t", bufs=2, space="PSUM") as ps1t:
            pend_t = None              # (tb4, vt_sb) awaiting transpose
            for tb4 in range(4):       # 512-wide t blocks
                tsl = slice(tb4 * 512, (tb4 + 1) * 512)
                q_ps = [ps1.tile([P, 512], F32, tag=f"qps{h}", name=f"qps{h}")
                        for h in range(HPC)]
                k_ps = ps1.tile([P, 512], F32, tag="kps")
                v_ps = ps1.tile([P, 512], F32, tag="vps")
                hsts = {}

                def kv_mm(k):
                    hst = hsts.pop(k)
                    st, sp = (k == 0), (k == NK - 1)
                    mm(k_ps[:], wk_sb[:, k, :], hst[:], st, sp)
                    mm(v_ps[:], wv_sb[:, k, :], hst[:], st, sp)

                # hst chunk boundaries: small first chunks so the first
                # matmuls start early, 1MB chunks after
                hchunks = {0: 2, 2: 2, 4: 4, 8: 4, 12: 4} if tb4 == 0 else \
                          {0: 4, 4: 4, 8: 4, 12: 4}
                for k in range(NK):
                    if tb4 == 0:
                        # weights stream on the ACT HWDGE ring, ordered
                        # just-in-time for the (kv-lagged) consumption
                        wsched = {
                            0: [(wq_sb, wqT_r, 0, 2), (wq_sb, wqT_r, 2, 4),
                                (wk_sb, wkT_r, 0, 4), (wv_sb, wvT_r, 0, 4)],
                            2: [(wq_sb, wqT_r, 4, 8)],
                            4: [(wk_sb, wkT_r, 4, 8), (wv_sb, wvT_r, 4, 8)],
                            5: [(wq_sb, wqT_r, 8, 12)],
                            8: [(wq_sb, wqT_r, 12, 16)],
                            9: [(wk_sb, wkT_r, 8, 12), (wv_sb, wvT_r, 8, 12)],
                            11: [(wk_sb, wkT_r, 12, 16), (wv_sb, wvT_r, 12, 16)],
                        }
                        for dst, srcp, a, b in wsched.get(k, []):
                            nc.scalar.dma_start(out=dst[:, a:b, :],
                                                in_=srcp[:, a:b, :])
                        if k == 11:
                            nc.scalar.dma_start(out=dmask, in_=dmd)
                            nc.scalar.dma_start(out=ones, in_=onesd)
                    elif tb4 in (1, 2) and k % 8 == 0:
                        # out-proj weights, needed from phase 2 on
                        h = (tb4 - 1) * 2 + k // 8
                        nc.scalar.dma_start(out=wo_sb[:, h, :], in_=woT_r[:, h, :])
                    if k in hchunks:
                        # batched hst chunks: per-DMA fixed costs serialize
                        # on the HWDGE ring, so per-slice DMAs cap it at
                        # ~200GB/s
                        n = hchunks[k]
                        hst4 = hpool.tile([P, 4, 512], F32R)
                        nc.sync.dma_start(out=hst4[:, :n, :],
                                          in_=hsT_r[:, k:k + n, tsl])
                        for k4 in range(n):
                            hsts[k + k4] = hst4[:, k4, :]
                    hst = hsts[k]
                    st, sp = (k == 0), (k == NK - 1)
                    for h in range(HPC):
                        mm(q_ps[h][:], wq_sb[:, k, h * D:(h + 1) * D], hst, st, sp)
                    if k == 2 and pend_t is not None:
                        transpose_v(*pend_t)
                        pend_t = None
                    if k >= KVLAG:
                        kv_mm(k - KVLAG)
                for k in range(NK - KVLAG, NK):
                    kv_mm(k)
                # evacuate PSUM: two q copies via ACT to parallelize
                vt_sb = spool.tile([P, 512], F32, tag="vt")
                if tb4 == 3:  # last block: vt first, transpose immediately
                    nc.vector.tensor_copy(vt_sb[:], v_ps[:])
                    transpose_v(tb4, vt_sb)
                nc.scalar.copy(qt_sb[:, 0, tsl], q_ps[0][:])
                nc.scalar.copy(qt_sb[:, 1, tsl], q_ps[1][:])
                nc.vector.tensor_copy(qt_sb[:, 2, tsl], q_ps[2][:])
                nc.vector.tensor_copy(qt_sb[:, 3, tsl], q_ps[3][:])
                nc.vector.tensor_copy(kt_sb[:, tsl], k_ps[:])
                if tb4 < 3:
                    nc.vector.tensor_copy(vt_sb[:], v_ps[:])
                    pend_t = (tb4, vt_sb)

        # ---------------- Phase 2: causal attention, 4 heads at once ----
        # S^T tile per (query 128-block tb, key tile st<=tb):
        #   [s=128, (h=4, t=128)] = lhsT(K^T s-tile) @ rhs(Q^T all heads)
        dmask_b = dmask[:, None, :].to_broadcast([P, HPC, P])

        def outproj(tb, at_t, ps3):
            for jb in range(4):
                jsl = slice(jb * 512, (jb + 1) * 512)
                op_ps = ps3.tile([P, 512], F32, tag="op")
                for h in range(HPC):
                    mm(op_ps[:], at_t[:, h, :], wo_sb[:, h, jsl],
                       h == 0, h == HPC - 1)
                oto = opool.tile([P, 512], F32, tag="oto")
                nc.vector.tensor_copy(oto[:], op_ps[:])
                nc.sync.dma_start(out=out_r[tb][:, jsl], in_=oto[:])

        with tc.tile_pool(name="ps2s", bufs=3, space="PSUM") as ps2s, \
             tc.tile_pool(name="ps2o", bufs=2, space="PSUM") as ps2o, \
             tc.tile_pool(name="ps2d", bufs=1, space="PSUM") as ps2d, \
             tc.tile_pool(name="ps3", bufs=2, space="PSUM") as ps3:
            # ---- fused prologue: tiles 0-2 interleaved, so their den/ot
            # matmuls never wait on ACT exp latency ----
            pstate = {}

            def p_new(tb):
                pstate[tb] = {
                    "ot": ps2o.tile([P, HPC, P], F32, tag="ot",
                                    name=f"pot{tb}"),
                    "den": ps2d.tile([P, HPC, P], F32, tag="den",
                                     name=f"pden{tb}"),
                    "e": {}}

            def p_score(tb, st):
                s_ps = ps2s.tile([P, HPC, P], F32, tag="sps")
                mm(s_ps[:], kt_sb[:, st * P:(st + 1) * P],
                   qt_sb[:, :, tb * P:(tb + 1) * P], True, True)
                e_sb = epool.tile([P, HPC, P], F32R, tag="etile")
                nc.scalar.activation(e_sb[:], s_ps[:], EXP)
                if st == tb:
                    nc.vector.tensor_mul(e_sb[:], e_sb[:], dmask_b)
                pstate[tb]["e"][st] = e_sb

            def p_denot(tb, st, first, last):
                e_sb = pstate[tb]["e"].pop(st)
                mm(pstate[tb]["den"][:], ones[:], e_sb[:], first, last)
                mm(pstate[tb]["ot"][:], v_sb[:, st, :], e_sb[:], first, last)

            def p_finish(tb):
                recip = spool.tile([P, HPC, P], F32, tag="recip")
                nc.vector.reciprocal_approx_fast(out=recip[:],
                                                 in_=pstate[tb]["den"][:])
                at_t = apool.tile([P, HPC, P], F32R, tag="att")
                nc.vector.tensor_mul(at_t[:], pstate[tb]["ot"][:], recip[:])
                del pstate[tb]
                return at_t

            p_new(0); p_score(0, 0)
            p_new(1); p_score(1, 0); p_score(1, 1)
            p_denot(0, 0, True, True)
            at0 = p_finish(0)
            p_new(2); p_score(2, 0); p_score(2, 1)
            p_denot(1, 0, True, False); p_denot(1, 1, False, True)
            at1 = p_finish(1)
            p_score(2, 2)
            p_denot(2, 0, True, False); p_denot(2, 1, False, False)
            p_denot(2, 2, False, True)
            at2 = p_finish(2)
            outproj(0, at0, ps3)
            outproj(1, at1, ps3)

            prev = (2, at2)  # (tb, at-tile) pending out-projection
            for tb in range(3, KT):
                tsl = slice(tb * P, (tb + 1) * P)
                qrhs = qt_sb[:, :, tsl]              # [128, 4, 128]
                ot_ps = ps2o.tile([P, HPC, P], F32, tag="ot")
                den_ps = ps2d.tile([P, HPC, P], F32, tag="den")

                # score emission order: [0, 1, diag, 2, .., tb-1];
                # den/ot order: [0, 1, .., tb-1, diag] (diag last, so the
                # exp+mask chain has the whole loop to finish)
                s_order = [0, 1, tb, *range(2, tb)] if tb >= 2 else \
                          list(range(tb + 1))
                d_order = [*range(tb), tb]
                e_tiles = {}

                def den_ot(st):
                    e_sb = e_tiles.pop(st)
                    first, last = (st == d_order[0]), (st == d_order[-1])
                    mm(den_ps[:], ones[:], e_sb[:], first, last)
                    mm(ot_ps[:], v_sb[:, st, :], e_sb[:], first, last)

                done = 0
                for idx, st in enumerate(s_order):
                    s_ps = ps2s.tile([P, HPC, P], F32, tag="sps")
                    mm(s_ps[:], kt_sb[:, st * P:(st + 1) * P], qrhs, True, True)
                    e_sb = epool.tile([P, HPC, P], F32R, tag="etile")
                    nc.scalar.activation(e_sb[:], s_ps[:], EXP)
                    if st == tb:  # diagonal tile: causal mask
                        nc.vector.tensor_mul(e_sb[:], e_sb[:], dmask_b)
                    e_tiles[st] = e_sb
                    if idx >= 3:
                        den_ot(d_order[done])
                        done += 1
                while done < len(d_order):
                    den_ot(d_order[done])
                    done += 1

                recip = spool.tile([P, HPC, P], F32, tag="recip")
                nc.vector.reciprocal_approx_fast(out=recip[:], in_=den_ps[:])
                at_t = apool.tile([P, HPC, P], F32R, tag="att")
                nc.vector.tensor_mul(at_t[:], ot_ps[:], recip[:])
                if prev is not None:
                    outproj(*prev, ps3)
                prev = (tb, at_t)
            outproj(*prev, ps3)

    nc.compile()
    return nc


_CACHE = {}


def _get_program():
    if "nc" not in _CACHE:
        _CACHE["nc"] = _build_program()
    return _CACHE["nc"]


def _host_inputs(hidden_states, Wq, Wk, Wv, Wo):
    """Fold rope+scale into weights, build per-core input maps."""
    f64 = np.float64
    mats = _rope_fold()
    scale = D ** -0.5
    Wq_f = np.empty((HID, HID), dtype=np.float32)
    for h in range(H):
        Wq_f[h * D:(h + 1) * D] = (mats[h] @ Wq[h * D:(h + 1) * D].astype(f64)
                                   * scale).astype(np.float32)
    perm = np.concatenate([np.arange(0, 64, 2), np.arange(1, 64, 2),
                           np.arange(64, 128, 2), np.arange(65, 128, 2)])
    Wk_f = Wk[perm].astype(np.float32)

    wkT = np.ascontiguousarray(Wk_f.T)
    wvT = np.ascontiguousarray(Wv.T)
    ii = np.arange(P)[:, None]
    jj = np.arange(P)[None, :]
    dmask = (ii <= jj).astype(np.float32)

    hsT = [np.ascontiguousarray(hidden_states[b].T) for b in range(B)]
    in_maps = []
    for c in range(NCORES):
        b, q = c // CPB, c % CPB
        rows = slice(q * HD_PC, (q + 1) * HD_PC)
        in_maps.append({
            "hsT": hsT[b],
            "wqT": np.ascontiguousarray(Wq_f[rows].T),
            "wkT": wkT,
            "wvT": wvT,
            "woT": np.ascontiguousarray(Wo[:, rows].T),
            "dmask": dmask,
            "onesd": np.ones((P, P), dtype=np.float32),
        })
    return in_maps


def kernel(hidden_states, Wq, Wk, Wv, Wo):
    hidden_states = np.asarray(hidden_states, dtype=np.float32)
    Wq = np.asarray(Wq, dtype=np.float32)
    Wk = np.asarray(Wk, dtype=np.float32)
    Wv = np.asarray(Wv, dtype=np.float32)
    Wo = np.asarray(Wo, dtype=np.float32)

    nc = _get_program()
    in_maps = _host_inputs(hidden_states, Wq, Wk, Wv, Wo)
    res = run_bass_kernel_spmd(nc, in_maps, list(range(NCORES)))
    parts = [r["out"] for r in res.results]
    out = np.empty((B, T, HID), dtype=np.float32)
    for b in range(B):
        out[b] = parts[CPB * b]
        for q in range(1, CPB):
            out[b] += parts[CPB * b + q]
    return out


# revision 17
# speedup vs baseline: 1.0101x; 1.0101x over previous
"""MQA kernel for Trainium2 (8 NeuronCores, SPMD via bass/Tile).

Problem: nn_MultiQueryAttention (B=2, T=2048, HID=2048, H=16, D=128).

Key algebraic simplification: the reference's apply_rope treats q's layout
as (B,T,H,D) while q is actually (B,H,T,D), so the "position" axis is the
head index -> per-head rotation R_h acting on the D axis only, independent
of sequence position. R_h is folded into Wq on the host. k's rope at pos=0
is a pure channel permutation, folded into Wk. The score scale 1/sqrt(D)
is folded into Wq as well. What remains on-device is a plain causal MQA.

Sharding (uniform SPMD program, per-core data differs):
  core c -> batch c//4, heads (c%4)*4..(c%4)*4+3, full T.
  Each core: Q^T/K^T/V projections, causal softmax attention for its 4
  heads, and a partial out-projection (its heads' rows of Wo^T). The 4
  partials per batch are summed on the host.

All matmuls run as float32r (full fp32 data, fast PE mode), fp32 PSUM
accumulation. bf16 was tried and is a net loss on this part: the
compiler-automatic FWL weight loads contend with the streaming operand
(matmul spacing 259ns vs 231ns) and ACT/DVE slow down ~20%.

Structure (vs the 348us baseline; measured 284.6us):
  - Softmax denominator matmul uses a [128,128] all-ones lhsT so the
    denominator lands replicated across partitions: the rank-1 broadcast
    matmul and the 1-partition DVE chain disappear.
  - Attention for query tile tb emits scores in order [0, 1, diag, 2,
    ...], den/ot matmuls two behind, diag den/ot last: the diagonal
    exp+mask chain and the ACT latency hide under the score stream.
  - Out-projection for tile tb is emitted after tile tb+1's attention
    loop, so the recip/normalize DVE chain hides under out-proj matmuls.
  - DMA: hst arrives in batched chunks on the sync HWDGE ring while
    weights stream just-in-time on the ACT HWDGE ring (per-DMA fixed
    costs serialize within a ring, capping many-small-DMA streams at
    ~200GB/s); K/V projection matmuls lag the Q matmuls by 3 k-steps so
    the wk/wv arrival is off the critical path; per-block V-transposes
    are deferred into the next block's k-loop; Wo loads spread across
    t-blocks 1-2; exp activation table preloaded during phase 1.
"""

import numpy as np
from contextlib import ExitStack

import concourse.bass as bass
import concourse.tile as tile
from concourse import bacc, mybir
from concourse.bass_utils import run_bass_kernel_spmd
from concourse.masks import make_identity

F32 = mybir.dt.float32
F32R = mybir.dt.float32r
EXP = mybir.ActivationFunctionType.Exp

B, T, HID, H, D = 2, 2048, 2048, 16, 128
NCORES = 8
CPB = 4              # cores per batch
HPC = H // CPB       # 4 heads per core
HD_PC = HPC * D      # 512 output dims per core
P = 128
KT = T // P          # 16 key tiles
NK = HID // P        # 16 contraction tiles for projections


def _rope_fold():
    """Per-head rotation matrices R_h (128x128) from the reference's quirky rope."""
    half = D // 2
    theta = 1.0 / (10000.0 ** (np.arange(0, half, 2, dtype=np.float64) / half))
    mats = []
    for h in range(H):
        R = np.zeros((D, D), dtype=np.float64)
        c = np.cos(h * theta)
        s = np.sin(h * theta)
        for j in range(32):
            R[j, 2 * j] = c[j]
            R[j, 2 * j + 1] = -s[j]
            R[32 + j, 2 * j] = s[j]
            R[32 + j, 2 * j + 1] = c[j]
            R[64 + j, 64 + 2 * j] = c[j]
            R[64 + j, 64 + 2 * j + 1] = -s[j]
            R[96 + j, 64 + 2 * j] = s[j]
            R[96 + j, 64 + 2 * j + 1] = c[j]
        mats.append(R)
    return mats


def _build_program():
    nc = bacc.Bacc("TRN2", target_bir_lowering=False, debug=False,
                   enable_asserts=False, num_devices=NCORES)

    hsT = nc.dram_tensor("hsT", [HID, T], F32R, kind="ExternalInput").ap()
    wqT = nc.dram_tensor("wqT", [HID, HD_PC], F32R, kind="ExternalInput").ap()
    wkT = nc.dram_tensor("wkT", [HID, D], F32R, kind="ExternalInput").ap()
    wvT = nc.dram_tensor("wvT", [HID, D], F32R, kind="ExternalInput").ap()
    woT = nc.dram_tensor("woT", [HD_PC, HID], F32R, kind="ExternalInput").ap()
    dmd = nc.dram_tensor("dmask", [P, P], F32R, kind="ExternalInput").ap()
    onesd = nc.dram_tensor("onesd", [P, P], F32R, kind="ExternalInput").ap()
    out = nc.dram_tensor("out", [T, HID], F32, kind="ExternalOutput").ap()

    hsT_r = hsT.rearrange("(ko p) t -> p ko t", p=P)        # [128,16,2048]
    wqT_r = wqT.rearrange("(ko p) m -> p ko m", p=P)        # [128,16,512]
    wkT_r = wkT.rearrange("(ko p) d -> p ko d", p=P)        # [128,16,128]
    wvT_r = wvT.rearrange("(ko p) d -> p ko d", p=P)
    woT_r = woT.rearrange("(h p) n -> p h n", p=P)          # [128,4,2048]
    out_r = out.rearrange("(tt p) n -> tt p n", p=P)        # [16,128,2048]

    def mm(ps, lhsT, rhs, start, stop):
        nc.tensor.matmul(ps, lhsT=lhsT, rhs=rhs, start=start, stop=stop)

    with tile.TileContext(nc) as tc, ExitStack() as ctx:
        singles = ctx.enter_context(tc.tile_pool(name="singles", bufs=1))
        hpool = ctx.enter_context(tc.tile_pool(name="hst", bufs=3))
        epool = ctx.enter_context(tc.tile_pool(name="etile", bufs=6))
        spool = ctx.enter_context(tc.tile_pool(name="small", bufs=2))
        apool = ctx.enter_context(tc.tile_pool(name="att", bufs=3))
        opool = ctx.enter_context(tc.tile_pool(name="outt", bufs=3))

        ident = singles.tile([P, P], F32)
        make_identity(nc, ident)
        dmask = singles.tile([P, P], F32R)
        ones = singles.tile([P, P], F32R)
        # preload the exp activation table while phase 1 streams
        warm = spool.tile([1, 1], F32, tag="warm")
        nc.scalar.activation(warm[:], ident[:1, :1], EXP)

        # weight residents; per-k slices are DMA'd inside the first
        # phase-1 block so the first matmuls start after ~1us; the
        # out-projection weights load during blocks 1-2.
        wq_sb = singles.tile([P, NK, HD_PC], F32R)
        wk_sb = singles.tile([P, NK, D], F32R)
        wv_sb = singles.tile([P, NK, D], F32R)
        wo_sb = singles.tile([P, HPC, HID], F32R)

        # resident activations
        qt_sb = singles.tile([P, HPC, T], F32R)      # Q^T per head [d, t]
        kt_sb = singles.tile([P, T], F32R)           # K^T [d, s]
        v_sb = singles.tile([P, KT, D], F32R)        # V natural [s-tile, d]

        # ---------------- Phase 1: Q/K/V projections ----------------
        # K/V matmuls run 3 k-steps behind the Q matmuls so the initial
        # wk/wv weight DMAs are off the critical path; each block's
        # V-transposes are deferred into the next block's k-loop.
        KVLAG = 3

        def transpose_v(tb4, vt):
            for si in range(4):
                pt = ps1t.tile([P, P], F32, tag="tps")
                nc.tensor.transpose(pt[:], vt[:, si * P:(si + 1) * P], ident[:])
                nc.vector.tensor_copy(v_sb[:, tb4 * 4 + si, :], pt[:])

        with tc.tile_pool(name="ps1", bufs=1, space="PSUM") as ps1, \
             tc.tile_pool(name="ps1t", bufs=2, space="PSUM") as ps1t:
            pend_t = None              # (tb4, vt_sb) awaiting transpose
            for tb4 in range(4):       # 512-wide t blocks
                tsl = slice(tb4 * 512, (tb4 + 1) * 512)
                q_ps = [ps1.tile([P, 512], F32, tag=f"qps{h}", name=f"qps{h}")
                        for h in range(HPC)]
                k_ps = ps1.tile([P, 512], F32, tag="kps")
                v_ps = ps1.tile([P, 512], F32, tag="vps")
                hsts = {}

                def kv_mm(k):
                    hst = hsts.pop(k)
                    st, sp = (k == 0), (k == NK - 1)
                    mm(k_ps[:], wk_sb[:, k, :], hst[:], st, sp)
                    mm(v_ps[:], wv_sb[:, k, :], hst[:], st, sp)

                # hst chunk boundaries: small first chunks so the first
                # matmuls start early, 1MB chunks after
                hchunks = {0: 2, 2: 2, 4: 4, 8: 4, 12: 4} if tb4 == 0 else \
                          {0: 4, 4: 4, 8: 4, 12: 4}
                for k in range(NK):
                    if tb4 == 0:
                        # weights stream on the ACT HWDGE ring, ordered
                        # just-in-time for the (kv-lagged) consumption
                        if k == 0:
                            nc.scalar.dma_start(out=wq_sb[:, 0:1, :],
                                                in_=wqT_r[:, 0:1, :])
                            nc.scalar.dma_start(out=wq_sb[:, 1:2, :],
                                                in_=wqT_r[:, 1:2, :])
                            nc.scalar.dma_start(out=wk_sb[:, 0:8, :],
                                                in_=wkT_r[:, 0:8, :])
                            nc.scalar.dma_start(out=wv_sb[:, 0:8, :],
                                                in_=wvT_r[:, 0:8, :])
                            nc.scalar.dma_start(out=wq_sb[:, 2:4, :],
                                                in_=wqT_r[:, 2:4, :])
                        elif k == 2:
                            nc.scalar.dma_start(out=wk_sb[:, 8:16, :],
                                                in_=wkT_r[:, 8:16, :])
                            nc.scalar.dma_start(out=wv_sb[:, 8:16, :],
                                                in_=wvT_r[:, 8:16, :])
                        elif k in (4, 8, 12):
                            ksl = slice(k, k + 4)
                            nc.scalar.dma_start(out=wq_sb[:, ksl, :],
                                                in_=wqT_r[:, ksl, :])
                        if k == 8:
                            nc.scalar.dma_start(out=dmask, in_=dmd)
                            nc.scalar.dma_start(out=ones, in_=onesd)
                    elif tb4 in (1, 2) and k % 8 == 0:
                        # out-proj weights, needed from phase 2 on
                        h = (tb4 - 1) * 2 + k // 8
                        nc.scalar.dma_start(out=wo_sb[:, h, :], in_=woT_r[:, h, :])
                    if k in hchunks:
                        # batched hst chunks: per-DMA fixed costs serialize
                        # on the HWDGE ring, so per-slice DMAs cap it at
                        # ~200GB/s
                        n = hchunks[k]
                        hst4 = hpool.tile([P, 4, 512], F32R)
                        nc.sync.dma_start(out=hst4[:, :n, :],
                                          in_=hsT_r[:, k:k + n, tsl])
                        for k4 in range(n):
                            hsts[k + k4] = hst4[:, k4, :]
                    hst = hsts[k]
                    st, sp = (k == 0), (k == NK - 1)
                    for h in range(HPC):
                        mm(q_ps[h][:], wq_sb[:, k, h * D:(h + 1) * D], hst, st, sp)
                    if k == 2 and pend_t is not None:
                        transpose_v(*pend_t)
                        pend_t = None
                    if k >= KVLAG:
                        kv_mm(k - KVLAG)
                for k in range(NK - KVLAG, NK):
                    kv_mm(k)
                # evacuate PSUM: two q copies via ACT to parallelize
                vt_sb = spool.tile([P, 512], F32, tag="vt")
                if tb4 == 3:  # last block: vt first, transpose immediately
                    nc.vector.tensor_copy(vt_sb[:], v_ps[:])
                    transpose_v(tb4, vt_sb)
                nc.scalar.copy(qt_sb[:, 0, tsl], q_ps[0][:])
                nc.scalar.copy(qt_sb[:, 1, tsl], q_ps[1][:])
                nc.vector.tensor_copy(qt_sb[:, 2, tsl], q_ps[2][:])
                nc.vector.tensor_copy(qt_sb[:, 3, tsl], q_ps[3][:])
                nc.vector.tensor_copy(kt_sb[:, tsl], k_ps[:])
                if tb4 < 3:
                    nc.vector.tensor_copy(vt_sb[:], v_ps[:])
                    pend_t = (tb4, vt_sb)

        # ---------------- Phase 2: causal attention, 4 heads at once ----
        # S^T tile per (query 128-block tb, key tile st<=tb):
        #   [s=128, (h=4, t=128)] = lhsT(K^T s-tile) @ rhs(Q^T all heads)
        dmask_b = dmask[:, None, :].to_broadcast([P, HPC, P])

        def outproj(tb, at_t, ps3):
            for jb in range(4):
                jsl = slice(jb * 512, (jb + 1) * 512)
                op_ps = ps3.tile([P, 512], F32, tag="op")
                for h in range(HPC):
                    mm(op_ps[:], at_t[:, h, :], wo_sb[:, h, jsl],
                       h == 0, h == HPC - 1)
                oto = opool.tile([P, 512], F32, tag="oto")
                nc.vector.tensor_copy(oto[:], op_ps[:])
                nc.sync.dma_start(out=out_r[tb][:, jsl], in_=oto[:])

        with tc.tile_pool(name="ps2s", bufs=3, space="PSUM") as ps2s, \
             tc.tile_pool(name="ps2o", bufs=2, space="PSUM") as ps2o, \
             tc.tile_pool(name="ps2d", bufs=1, space="PSUM") as ps2d, \
             tc.tile_pool(name="ps3", bufs=2, space="PSUM") as ps3:
            prev = None  # (tb, at-tile) pending out-projection
            for tb in range(KT):
                tsl = slice(tb * P, (tb + 1) * P)
                qrhs = qt_sb[:, :, tsl]              # [128, 4, 128]
                ot_ps = ps2o.tile([P, HPC, P], F32, tag="ot")
                den_ps = ps2d.tile([P, HPC, P], F32, tag="den")

                # score emission order: [0, 1, diag, 2, .., tb-1];
                # den/ot order: [0, 1, .., tb-1, diag] (diag last, so the
                # exp+mask chain has the whole loop to finish)
                s_order = [0, 1, tb, *range(2, tb)] if tb >= 2 else \
                          list(range(tb + 1))
                d_order = [*range(tb), tb]
                e_tiles = {}

                def den_ot(st):
                    e_sb = e_tiles.pop(st)
                    first, last = (st == d_order[0]), (st == d_order[-1])
                    mm(den_ps[:], ones[:], e_sb[:], first, last)
                    mm(ot_ps[:], v_sb[:, st, :], e_sb[:], first, last)

                done = 0
                for idx, st in enumerate(s_order):
                    s_ps = ps2s.tile([P, HPC, P], F32, tag="sps")
                    mm(s_ps[:], kt_sb[:, st * P:(st + 1) * P], qrhs, True, True)
                    e_sb = epool.tile([P, HPC, P], F32R, tag="etile")
                    nc.scalar.activation(e_sb[:], s_ps[:], EXP)
                    if st == tb:  # diagonal tile: causal mask
                        nc.vector.tensor_mul(e_sb[:], e_sb[:], dmask_b)
                    e_tiles[st] = e_sb
                    if idx >= 3:
                        den_ot(d_order[done])
                        done += 1
                while done < len(d_order):
                    den_ot(d_order[done])
                    done += 1

                recip = spool.tile([P, HPC, P], F32, tag="recip")
                nc.vector.reciprocal_approx_fast(out=recip[:], in_=den_ps[:])
                at_t = apool.tile([P, HPC, P], F32R, tag="att")
                nc.vector.tensor_mul(at_t[:], ot_ps[:], recip[:])
                if prev is not None:
                    outproj(*prev, ps3)
                prev = (tb, at_t)
            outproj(*prev, ps3)

    nc.compile()
    return nc


_CACHE = {}


def _get_program():
    if "nc" not in _CACHE:
        _CACHE["nc"] = _build_program()
    return _CACHE["nc"]


def _host_inputs(hidden_states, Wq, Wk, Wv, Wo):
    """Fold rope+scale into weights, build per-core input maps."""
    f64 = np.float64
    mats = _rope_fold()
    scale = D ** -0.5
    Wq_f = np.empty((HID, HID), dtype=np.float32)
    for h in range(H):
        Wq_f[h * D:(h + 1) * D] = (mats[h] @ Wq[h * D:(h + 1) * D].astype(f64)
                                   * scale).astype(np.float32)
    perm = np.concatenate([np.arange(0, 64, 2), np.arange(1, 64, 2),
                           np.arange(64, 128, 2), np.arange(65, 128, 2)])
    Wk_f = Wk[perm].astype(np.float32)

    wkT = np.ascontiguousarray(Wk_f.T)
    wvT = np.ascontiguousarray(Wv.T)
    ii = np.arange(P)[:, None]
    jj = np.arange(P)[None, :]
    dmask = (ii <= jj).astype(np.float32)

    hsT = [np.ascontiguousarray(hidden_states[b].T) for b in range(B)]
    in_maps = []
    for c in range(NCORES):
        b, q = c // CPB, c % CPB
        rows = slice(q * HD_PC, (q + 1) * HD_PC)
        in_maps.append({
            "hsT": hsT[b],
            "wqT": np.ascontiguousarray(Wq_f[rows].T),
            "wkT": wkT,
            "wvT": wvT,
            "woT": np.ascontiguousarray(Wo[:, rows].T),
            "dmask": dmask,
            "onesd": np.ones((P, P), dtype=np.float32),
        })
    return in_maps


def kernel(hidden_states, Wq, Wk, Wv, Wo):
    hidden_states = np.asarray(hidden_states, dtype=np.float32)
    Wq = np.asarray(Wq, dtype=np.float32)
    Wk = np.asarray(Wk, dtype=np.float32)
    Wv = np.asarray(Wv, dtype=np.float32)
    Wo = np.asarray(Wo, dtype=np.float32)

    nc = _get_program()
    in_maps = _host_inputs(hidden_states, Wq, Wk, Wv, Wo)
    res = run_bass_kernel_spmd(nc, in_maps, list(range(NCORES)))
    parts = [r["out"] for r in res.results]
    out = np.empty((B, T, HID), dtype=np.float32)
    for b in range(B):
        out[b] = parts[CPB * b]
        for q in range(1, CPB):
            out[b] += parts[CPB * b + q]
    return out
